# revision 37
# baseline (speedup 1.0000x reference)
"""Attention (B=4, S=4096, W=512, E=64) on 8 TRN2 NeuronCores.

Sharding: core c handles batch b = c//2, query half h = c%2 (2048 queries).
Each core receives x[b]^T as bf16 with the key/value columns ordered so that
this core's query half occupies columns [0, 2048) (softmax over keys is
permutation invariant as long as K and V share the order, so odd cores get
the two halves swapped). K/V are computed for the full sequence locally; a
flash-style attention runs over the core's query half. No collectives
(pair-wise AllGather was measured at ~17us per op in this stack - slower
than just duplicating the K/V projection on both cores of a pair).

Per-core dataflow (bf16 scores, fp8e4 AV with DoubleRow):
  x^T [512,4096] --[Wv|Wk] pass--> kv = V^T (p0:64) / K^T (p64:128)
  x^T[:, :2048] --[Wq|Wq] pass--> Q^T duplicated on both partition halves
  K^T replicated to partitions 0:64 via SBUF->SBUF DMA on the sync
  queue (the SP engine is idle after the x loads; a gpsimd-queue DMA
  would cost a ~2us dge drain at teardown, and an Act-queue trigger
  would stall ScalarE on the bias-add semaphores)
  V' = [V | 1 | pad-to-80] tiles in fp8e4 via PE transpose, interleaved
  with the projection chunks to ride the x-DMA gaps
  scores: S^T[k,q] = K^T.T @ Q^T, two k-tiles run CONCURRENTLY in the two
  64-row PE row groups (~386ns per pair)
  P = exp(S^T/8) -> fp8e4, one full [128,2x512] instruction per k-pair,
  alternating engines: even kp native Exp on ScalarE, odd kp on VectorE
  via Schraudolph (bits = s*SCH_A + SCH_B stored int8; that bit pattern
  IS the e4m3 encoding of exp(s/8), since e4m3 has 8 codes per octave)
  Z'^T[e+1,q] += V'.T @ P as ONE fp8 DoubleRow matmul per k-pair (~405ns
  for 256 contraction rows; V' ones column accumulates the denominator)
  normalize per query chunk: PE-transpose Z'^T, reciprocal on VectorE,
  scale on ScalarE, per-sub DMA out - overlapped with the next sweep.

PSUM: projection pool 6 banks (closes before the loop; fewer buffers
starve the PE behind the bias-add/V'-copy round trips), then Z
accumulator 1 bank + norm scratch 1 bank + triple-buffered score pairs
6 banks.  AV matmuls are emitted three iterations late so the PE's
in-order stream never waits on exp; exp of pair k frees its PSUM banks
before the scores of pair k+3 need them.
"""

import numpy as np
import ml_dtypes

import concourse.bass as bass
import concourse.mybir as mybir
import concourse.tile as tile
from concourse import bacc
from concourse.bass import ts
from concourse.masks import make_identity
from concourse.bass_utils import run_bass_kernel_spmd

BF16 = mybir.dt.bfloat16
F32 = mybir.dt.float32
FP8 = mybir.dt.float8e4
INT8 = mybir.dt.int8
NP_BF16 = ml_dtypes.bfloat16

# Schraudolph exp on DVE: bits = s * SCH_A + SCH_B, stored int8, bitcast
# fp8e4.  SCH_A folds the 1/sqrt(E) softmax scale and log2(e) into the
# e4m3 exponent step (8 codes per octave); SCH_B centers on the exponent
# bias (7*8) minus the mean log error of the mantissa interpolation.
# Verified on hardware: the DVE float->int8 store rounds to nearest.
SCH_A = 0.125 * 8 * 1.4426950408889634
SCH_B = 56.0 - 0.458

B = 4
S_FULL = 4096
W = 512
E = 64
TQ = 2048  # queries per core
WT = W // 128  # 4 contraction tiles
KT = S_FULL // 128  # 32 key tiles
KP = KT // 2  # 16 key-tile pairs
QC = TQ // 512  # 4 query chunks of 512
NCH = S_FULL // 512  # 8 projection chunks
SCALE = 0.125  # 1/sqrt(E)

_NC_CACHE = {}


def build_nc():
    nc = bacc.Bacc("TRN2", target_bir_lowering=False)
    xT = nc.dram_tensor("xT", [W, S_FULL], BF16, kind="ExternalInput")
    wqq = nc.dram_tensor("wqq", [W, 128], BF16, kind="ExternalInput")
    wkv = nc.dram_tensor("wkv", [W, 128], BF16, kind="ExternalInput")
    bqq = nc.dram_tensor("bqq", [128, 1], F32, kind="ExternalInput")
    bkv = nc.dram_tensor("bkv", [128, 1], F32, kind="ExternalInput")
    y = nc.dram_tensor("y", [TQ, E], F32, kind="ExternalOutput")

    with tile.TileContext(nc) as tc:
        with (
            tc.tile_pool(name="const", bufs=1) as const,
            tc.tile_pool(name="psZ", bufs=1, space="PSUM") as psZ,
            tc.tile_pool(name="pp", bufs=8) as ppool,
            tc.tile_pool(name="zsb", bufs=2) as zsbp,
            tc.tile_pool(name="small", bufs=2) as small,
            tc.tile_pool(name="outp", bufs=2) as outp,
        ):
            # weights/biases as single HWDGE DMAs ahead of the x^T stream
            wqq_sb = const.tile([128, WT, 128], BF16)
            wkv_sb = const.tile([128, WT, 128], BF16)
            nc.scalar.dma_start(
                out=wkv_sb, in_=wkv[:, :].rearrange("(t p) m -> p t m", t=WT)
            )
            nc.scalar.dma_start(
                out=wqq_sb, in_=wqq[:, :].rearrange("(t p) m -> p t m", t=WT)
            )
            bqq_sb = const.tile([128, 1], F32)
            bkv_sb = const.tile([128, 1], F32)
            nc.scalar.dma_start(out=bkv_sb, in_=bkv[:, :])
            nc.scalar.dma_start(out=bqq_sb, in_=bqq[:, :])

            # x^T streamed in per 1024-column block (HWDGE)
            xt_sb = const.tile([128, WT, S_FULL], BF16)
            for ch2 in range(NCH // 2):
                for t in range(WT):
                    nc.sync.dma_start(
                        out=xt_sb[:, t, ts(ch2, 1024)],
                        in_=xT[t * 128:(t + 1) * 128, ts(ch2, 1024)],
                    )

            ident_bf = const.tile([64, 64], BF16)
            make_identity(nc, ident_bf)
            ident_f32 = const.tile([E + 1, E + 1], F32)
            make_identity(nc, ident_f32)

            kv_sb = const.tile([128, S_FULL], BF16)  # V^T (p0:64) / K^T (p64:)
            krep = const.tile([64, S_FULL], BF16)  # K^T replica on p0:64
            qtpair = const.tile([128, TQ], BF16)  # Q^T on both halves
            # inner dim padded 65->80 so the DoubleRow pair stride is
            # 16B-aligned
            vp_sb = const.tile([128, KT, 80], FP8)  # V' = [V | 1 | pad]
            nc.vector.memset(vp_sb, 1.0)

            with tc.tile_pool(name="psA", bufs=6, space="PSUM") as psA:
                def emit_kv_proj(ch):
                    # K/V projection chunk; one fused bias add on VectorE
                    ps = psA.tile([128, 512], F32, tag="mm", name=f"pskv{ch}")
                    for t in range(WT):
                        nc.tensor.matmul(
                            ps,
                            wkv_sb[:, t, :],
                            xt_sb[:, t, ts(ch, 512)],
                            start=(t == 0),
                            stop=(t == WT - 1),
                        )
                    nc.vector.tensor_scalar_add(
                        kv_sb[:, ts(ch, 512)], ps, bkv_sb
                    )
                    nc.sync.dma_start(
                        out=krep[:, ts(ch, 512)], in_=kv_sb[64:128, ts(ch, 512)]
                    )

                def emit_vtrans1(kt_i):
                    vt_ps = psA.tile(
                        [128, E], BF16, tag="mm", name=f"vtps{kt_i}"
                    )
                    nc.tensor.transpose(
                        vt_ps, kv_sb[0:64, ts(kt_i, 128)], ident_bf
                    )
                    # PSUM bf16 -> SBUF fp8; alternate engines to balance
                    if kt_i % 2 == 0:
                        nc.vector.tensor_copy(vp_sb[:, kt_i, 0:E], vt_ps)
                    else:
                        nc.scalar.copy(vp_sb[:, kt_i, 0:E], vt_ps)

                def emit_q_chunk(ch):
                    psq = psA.tile([128, 512], F32, tag="mm", name=f"psq{ch}")
                    for t in range(WT):
                        nc.tensor.matmul(
                            psq,
                            wqq_sb[:, t, :],
                            xt_sb[:, t, ts(ch, 512)],
                            start=(t == 0),
                            stop=(t == WT - 1),
                        )
                    # Q bias on ScalarE: DVE carries the K/V biases and
                    # fp8 copies, so its projection tail otherwise delays
                    # qc0's first Schraudolph exps
                    nc.scalar.activation(
                        qtpair[:, ts(ch, 512)], psq,
                        mybir.ActivationFunctionType.Identity, bias=bqq_sb,
                    )

                emit_kv_proj(0)
                emit_q_chunk(0)
                emit_kv_proj(1)
                for kt_i in range(0, 4):
                    emit_vtrans1(kt_i)
                emit_q_chunk(1)
                emit_kv_proj(2)
                for kt_i in range(4, 8):
                    emit_vtrans1(kt_i)
                emit_q_chunk(2)
                emit_kv_proj(3)
                for kt_i in range(8, 12):
                    emit_vtrans1(kt_i)
                emit_q_chunk(3)
                for ch in range(4, NCH):
                    emit_kv_proj(ch)
                    for kt_i in range(4 * ch - 4, 4 * ch):
                        emit_vtrans1(kt_i)
                for kt_i in range(28, 32):
                    emit_vtrans1(kt_i)

            with (
                tc.tile_pool(name="psB", bufs=3, space="PSUM") as psB,
                tc.tile_pool(name="psN", bufs=1, space="PSUM") as psN,
            ):
                norm_state = {}

                def norm_start(qc, zp):
                    zsb = zsbp.tile(
                        [E + 1, 512], F32, tag="zsb", name=f"zsb{qc}"
                    )
                    nc.vector.tensor_copy(zsb, zp)
                    o_sb = outp.tile([128, 4, E], F32, tag="o", name=f"osb{qc}")
                    norm_state[qc] = (zsb, o_sb)

                def norm_step(qc, sub, pool, spread=False):
                    zsb, o_sb = norm_state[qc]
                    zt = pool.tile(
                        [128, E + 1], F32, tag="zt", name=f"zt{qc}_{sub}"
                    )
                    nc.tensor.transpose(zt, zsb[:, ts(sub, 128)], ident_f32)
                    r = small.tile([128, 1], F32, tag="r", name=f"r{qc}_{sub}")
                    nc.vector.reciprocal(r, zt[:, E:E + 1])
                    # in-loop steps land at kps 3,6,9,12; kps 6 and 12
                    # (subs 1,3) are Act-exp kps, so their scale-mul goes
                    # to the idle VectorE instead
                    if sub % 2 == 1:
                        nc.vector.tensor_scalar_mul(
                            o_sb[:, sub, :], zt[:, 0:E], r
                        )
                    else:
                        nc.scalar.mul(o_sb[:, sub, :], zt[:, 0:E], r)
                    q_eng = (nc.sync, nc.scalar, nc.sync, nc.scalar)[
                        sub if spread else 0
                    ]
                    q_eng.dma_start(
                        out=y[ts(4 * qc + sub, 128), :], in_=o_sb[:, sub, :]
                    )

                prev = None  # (qc, zp) awaiting normalize
                for qc in range(QC):
                    zp = psZ.tile(
                        [E + 1, 512], F32, tag="zacc", name=f"zacc{qc}"
                    )
                    pending = []  # (p_tile, ka, kb) AVs deferred two iters
                    for kp in range(KP):
                        if prev is not None:
                            if kp == 0:
                                norm_start(*prev)
                            elif kp in (3, 6, 9, 12):
                                norm_step(prev[0], kp // 3 - 1, psN)
                        ka, kb = 2 * kp, 2 * kp + 1
                        sp = psB.tile(
                            [128, 2, 512], F32, tag="spair", name=f"sp{qc}_{kp}"
                        )
                        nc.tensor.matmul(
                            sp[:, 0, :],
                            krep[:, ts(ka, 128)],
                            qtpair[0:64, ts(qc, 512)],
                            start=True,
                            stop=True,
                        )
                        nc.tensor.matmul(
                            sp[:, 1, :],
                            kv_sb[64:128, ts(kb, 128)],
                            qtpair[64:128, ts(qc, 512)],
                            start=True,
                            stop=True,
                        )
                        p_sb = ppool.tile(
                            [128, 2, 512], FP8, tag="p", name=f"p{qc}_{kp}"
                        )
                        # full-pair exp alternates engines: ScalarE native
                        # Exp on even kp, VectorE Schraudolph on odd kp.
                        # kps 0-2 all go to ScalarE: it is idle at the qc
                        # boundary (DVE drains the Z-evacuation copy), and
                        # an early exp(kp0) completion unblocks the psB
                        # recycle that otherwise stalls the kp3 scores.
                        if kp < 3 or kp % 2 == 0:
                            nc.scalar.activation(
                                p_sb[:, :, :], sp[:, :, :],
                                mybir.ActivationFunctionType.Exp, scale=SCALE,
                            )
                        else:
                            nc.vector.tensor_scalar(
                                p_sb[:, :, :].bitcast(INT8), sp[:, :, :],
                                SCH_A, SCH_B,
                                op0=mybir.AluOpType.mult,
                                op1=mybir.AluOpType.add,
                            )
                        if len(pending) == 3:
                            pp_, pka, pkb = pending.pop(0)
                            nc.tensor.matmul(
                                zp, vp_sb[:, pka:pka + 2, 0:E + 1],
                                pp_[:, :, :],
                                start=(pka == 0), stop=False,
                                perf_mode=mybir.MatmulPerfMode.DoubleRow,
                            )
                        pending.append((p_sb, ka, kb))
                    for pp_, pka, pkb in pending:
                        nc.tensor.matmul(
                            zp, vp_sb[:, pka:pka + 2, 0:E + 1], pp_[:, :, :],
                            start=False, stop=(pkb == KT - 1),
                            perf_mode=mybir.MatmulPerfMode.DoubleRow,
                        )
                    prev = (qc, zp)
                # final chunk's normalize in the tail
                norm_start(*prev)
            with tc.tile_pool(name="psT", bufs=4, space="PSUM") as psT:
                for sub in range(4):
                    norm_step(prev[0], sub, psT, spread=True)
    nc.compile()
    return nc


def get_nc():
    if "nc" not in _NC_CACHE:
        _NC_CACHE["nc"] = build_nc()
    return _NC_CACHE["nc"]


def make_in_maps(x, Wq, bq, Wk, bk, Wv, bv):
    x = np.asarray(x, dtype=np.float32)
    Wq = np.asarray(Wq, dtype=np.float32)
    Wk = np.asarray(Wk, dtype=np.float32)
    Wv = np.asarray(Wv, dtype=np.float32)
    bq = np.asarray(bq, dtype=np.float32)
    bk = np.asarray(bk, dtype=np.float32)
    bv = np.asarray(bv, dtype=np.float32)

    wkv_host = np.ascontiguousarray(
        np.concatenate([Wv.T, Wk.T], axis=1)
    ).astype(NP_BF16)
    wqq_host = np.ascontiguousarray(
        np.concatenate([Wq.T, Wq.T], axis=1)
    ).astype(NP_BF16)
    bkv_host = np.ascontiguousarray(
        np.concatenate([bv, bk]).reshape(128, 1)
    ).astype(np.float32)
    bqq_host = np.ascontiguousarray(
        np.concatenate([bq, bq]).reshape(128, 1)
    ).astype(np.float32)

    in_maps = []
    for c in range(8):
        b, h = c // 2, c % 2
        xT_b = np.asarray(x[b].T, dtype=NP_BF16)
        if h == 1:  # put this core's query half into columns [0, 2048)
            xT_b = np.concatenate([xT_b[:, TQ:], xT_b[:, :TQ]], axis=1)
        in_maps.append(
            {
                "xT": np.ascontiguousarray(xT_b),
                "wqq": wqq_host,
                "wkv": wkv_host,
                "bqq": bqq_host,
                "bkv": bkv_host,
            }
        )
    return in_maps


def assemble(results):
    out = np.empty((B, S_FULL, E), dtype=np.float32)
    for c in range(8):
        b, h = c // 2, c % 2
        out[b, h * TQ:(h + 1) * TQ, :] = results[c]["y"]
    return out


def kernel(x, Wq, bq, Wk, bk, Wv, bv, **_unused):
    in_maps = make_in_maps(x, Wq, bq, Wk, bk, Wv, bv)
    nc = get_nc()
    res = run_bass_kernel_spmd(nc, in_maps, core_ids=list(range(8)))
    return assemble(res.results)


# revision 38
# speedup vs baseline: 1.0101x; 1.0101x over previous
"""Attention (B=4, S=4096, W=512, E=64) on 8 TRN2 NeuronCores.

Sharding: core c handles batch b = c//2, query half h = c%2 (2048 queries).
Each core receives x[b]^T as bf16 with the key/value columns ordered so that
this core's query half occupies columns [0, 2048) (softmax over keys is
permutation invariant as long as K and V share the order, so odd cores get
the two halves swapped). K/V are computed for the full sequence locally; a
flash-style attention runs over the core's query half. No collectives
(pair-wise AllGather was measured at ~17us per op in this stack - slower
than just duplicating the K/V projection on both cores of a pair).

Per-core dataflow (bf16 scores, fp8e4 AV with DoubleRow):
  x^T [512,4096] --[Wv|Wk] pass--> kv = V^T (p0:64) / K^T (p64:128)
  x^T[:, :2048] --[Wq|Wq] pass--> Q^T duplicated on both partition halves
  K^T replicated to partitions 0:64 via SBUF->SBUF DMA on the sync
  queue (the SP engine is idle after the x loads; a gpsimd-queue DMA
  would cost a ~2us dge drain at teardown, and an Act-queue trigger
  would stall ScalarE on the bias-add semaphores)
  V' = [V | 1 | pad-to-80] tiles in fp8e4 via PE transpose, interleaved
  with the projection chunks to ride the x-DMA gaps
  scores: S^T[k,q] = K^T.T @ Q^T, two k-tiles run CONCURRENTLY in the two
  64-row PE row groups (~386ns per pair)
  P = exp(S^T/8) -> fp8e4, one full [128,2x512] instruction per k-pair,
  alternating engines: even kp native Exp on ScalarE, odd kp on VectorE
  via Schraudolph (bits = s*SCH_A + SCH_B stored int8; that bit pattern
  IS the e4m3 encoding of exp(s/8), since e4m3 has 8 codes per octave)
  Z'^T[e+1,q] += V'.T @ P as ONE fp8 DoubleRow matmul per k-pair (~405ns
  for 256 contraction rows; V' ones column accumulates the denominator)
  normalize per query chunk: PE-transpose Z'^T, reciprocal on VectorE,
  scale on ScalarE, per-sub DMA out - overlapped with the next sweep.

PSUM: projection pool 6 banks (closes before the loop; fewer buffers
starve the PE behind the bias-add/V'-copy round trips), then Z
accumulator 1 bank + norm scratch 1 bank + triple-buffered score pairs
6 banks.  AV matmuls are emitted three iterations late so the PE's
in-order stream never waits on exp; exp of pair k frees its PSUM banks
before the scores of pair k+3 need them.
"""

import numpy as np
import ml_dtypes

import concourse.bass as bass
import concourse.mybir as mybir
import concourse.tile as tile
from concourse import bacc
from concourse.bass import ts
from concourse.masks import make_identity
from concourse.bass_utils import run_bass_kernel_spmd

BF16 = mybir.dt.bfloat16
F32 = mybir.dt.float32
FP8 = mybir.dt.float8e4
INT8 = mybir.dt.int8
NP_BF16 = ml_dtypes.bfloat16

# Schraudolph exp on DVE: bits = s * SCH_A + SCH_B, stored int8, bitcast
# fp8e4.  SCH_A folds the 1/sqrt(E) softmax scale and log2(e) into the
# e4m3 exponent step (8 codes per octave); SCH_B centers on the exponent
# bias (7*8) minus the mean log error of the mantissa interpolation.
# Verified on hardware: the DVE float->int8 store rounds to nearest.
SCH_A = 0.125 * 8 * 1.4426950408889634
SCH_B = 56.0 - 0.458

B = 4
S_FULL = 4096
W = 512
E = 64
TQ = 2048  # queries per core
WT = W // 128  # 4 contraction tiles
KT = S_FULL // 128  # 32 key tiles
KP = KT // 2  # 16 key-tile pairs
QC = TQ // 512  # 4 query chunks of 512
NCH = S_FULL // 512  # 8 projection chunks
SCALE = 0.125  # 1/sqrt(E)

_NC_CACHE = {}


def build_nc():
    nc = bacc.Bacc("TRN2", target_bir_lowering=False)
    xT = nc.dram_tensor("xT", [W, S_FULL], BF16, kind="ExternalInput")
    wqq = nc.dram_tensor("wqq", [W, 128], BF16, kind="ExternalInput")
    wkv = nc.dram_tensor("wkv", [W, 128], BF16, kind="ExternalInput")
    bqq = nc.dram_tensor("bqq", [128, 1], F32, kind="ExternalInput")
    bkv = nc.dram_tensor("bkv", [128, 1], F32, kind="ExternalInput")
    y = nc.dram_tensor("y", [TQ, E], F32, kind="ExternalOutput")

    with tile.TileContext(nc) as tc:
        with (
            tc.tile_pool(name="const", bufs=1) as const,
            tc.tile_pool(name="psZ", bufs=1, space="PSUM") as psZ,
            tc.tile_pool(name="pp", bufs=8) as ppool,
            tc.tile_pool(name="zsb", bufs=2) as zsbp,
            tc.tile_pool(name="small", bufs=2) as small,
            tc.tile_pool(name="outp", bufs=2) as outp,
        ):
            # weights/biases as single HWDGE DMAs ahead of the x^T stream
            wqq_sb = const.tile([128, WT, 128], BF16)
            wkv_sb = const.tile([128, WT, 128], BF16)
            nc.scalar.dma_start(
                out=wkv_sb, in_=wkv[:, :].rearrange("(t p) m -> p t m", t=WT)
            )
            nc.scalar.dma_start(
                out=wqq_sb, in_=wqq[:, :].rearrange("(t p) m -> p t m", t=WT)
            )
            bqq_sb = const.tile([128, 1], F32)
            bkv_sb = const.tile([128, 1], F32)
            nc.scalar.dma_start(out=bkv_sb, in_=bkv[:, :])
            nc.scalar.dma_start(out=bqq_sb, in_=bqq[:, :])

            # x^T streamed in per 1024-column block (HWDGE)
            xt_sb = const.tile([128, WT, S_FULL], BF16)
            for ch2 in range(NCH // 2):
                for t in range(WT):
                    nc.sync.dma_start(
                        out=xt_sb[:, t, ts(ch2, 1024)],
                        in_=xT[t * 128:(t + 1) * 128, ts(ch2, 1024)],
                    )

            ident_bf = const.tile([64, 64], BF16)
            make_identity(nc, ident_bf)
            ident_f32 = const.tile([E + 1, E + 1], F32)
            make_identity(nc, ident_f32)

            kv_sb = const.tile([128, S_FULL], BF16)  # V^T (p0:64) / K^T (p64:)
            krep = const.tile([64, S_FULL], BF16)  # K^T replica on p0:64
            qtpair = const.tile([128, TQ], BF16)  # Q^T on both halves
            # inner dim padded 65->80 so the DoubleRow pair stride is
            # 16B-aligned
            vp_sb = const.tile([128, KT, 80], FP8)  # V' = [V | 1 | pad]
            nc.vector.memset(vp_sb, 1.0)

            with tc.tile_pool(name="psA", bufs=6, space="PSUM") as psA:
                def emit_kv_proj(ch):
                    # K/V projection chunk; one fused bias add on VectorE
                    ps = psA.tile([128, 512], F32, tag="mm", name=f"pskv{ch}")
                    for t in range(WT):
                        nc.tensor.matmul(
                            ps,
                            wkv_sb[:, t, :],
                            xt_sb[:, t, ts(ch, 512)],
                            start=(t == 0),
                            stop=(t == WT - 1),
                        )
                    nc.vector.tensor_scalar_add(
                        kv_sb[:, ts(ch, 512)], ps, bkv_sb
                    )
                    nc.sync.dma_start(
                        out=krep[:, ts(ch, 512)], in_=kv_sb[64:128, ts(ch, 512)]
                    )

                def emit_vtrans1(kt_i):
                    vt_ps = psA.tile(
                        [128, E], BF16, tag="mm", name=f"vtps{kt_i}"
                    )
                    nc.tensor.transpose(
                        vt_ps, kv_sb[0:64, ts(kt_i, 128)], ident_bf
                    )
                    # PSUM bf16 -> SBUF fp8; alternate engines to balance
                    if kt_i % 2 == 0:
                        nc.vector.tensor_copy(vp_sb[:, kt_i, 0:E], vt_ps)
                    else:
                        nc.scalar.copy(vp_sb[:, kt_i, 0:E], vt_ps)

                def emit_q_chunk(ch):
                    psq = psA.tile([128, 512], F32, tag="mm", name=f"psq{ch}")
                    for t in range(WT):
                        nc.tensor.matmul(
                            psq,
                            wqq_sb[:, t, :],
                            xt_sb[:, t, ts(ch, 512)],
                            start=(t == 0),
                            stop=(t == WT - 1),
                        )
                    # Q bias on ScalarE: DVE carries the K/V biases and
                    # fp8 copies, so its projection tail otherwise delays
                    # qc0's first Schraudolph exps
                    nc.scalar.activation(
                        qtpair[:, ts(ch, 512)], psq,
                        mybir.ActivationFunctionType.Identity, bias=bqq_sb,
                    )

                emit_kv_proj(0)
                emit_q_chunk(0)
                emit_kv_proj(1)
                for kt_i in range(0, 4):
                    emit_vtrans1(kt_i)
                emit_q_chunk(1)
                emit_kv_proj(2)
                for kt_i in range(4, 8):
                    emit_vtrans1(kt_i)
                emit_q_chunk(2)
                emit_kv_proj(3)
                for kt_i in range(8, 12):
                    emit_vtrans1(kt_i)
                emit_q_chunk(3)
                for ch in range(4, NCH):
                    emit_kv_proj(ch)
                    for kt_i in range(4 * ch - 4, 4 * ch):
                        emit_vtrans1(kt_i)
                for kt_i in range(28, 32):
                    emit_vtrans1(kt_i)

            with (
                tc.tile_pool(name="psB", bufs=3, space="PSUM") as psB,
                tc.tile_pool(name="psN", bufs=1, space="PSUM") as psN,
            ):
                norm_state = {}

                def norm_start(qc, zp):
                    zsb = zsbp.tile(
                        [E + 1, 512], F32, tag="zsb", name=f"zsb{qc}"
                    )
                    nc.vector.tensor_copy(zsb, zp)
                    o_sb = outp.tile([128, 4, E], F32, tag="o", name=f"osb{qc}")
                    norm_state[qc] = (zsb, o_sb)

                def norm_step(qc, sub, pool, spread=False):
                    zsb, o_sb = norm_state[qc]
                    zt = pool.tile(
                        [128, E + 1], F32, tag="zt", name=f"zt{qc}_{sub}"
                    )
                    nc.tensor.transpose(zt, zsb[:, ts(sub, 128)], ident_f32)
                    r = small.tile([128, 1], F32, tag="r", name=f"r{qc}_{sub}")
                    nc.vector.reciprocal(r, zt[:, E:E + 1])
                    if spread and sub % 2 == 1:
                        nc.vector.tensor_scalar_mul(
                            o_sb[:, sub, :], zt[:, 0:E], r
                        )
                    else:
                        nc.scalar.mul(o_sb[:, sub, :], zt[:, 0:E], r)
                    q_eng = (nc.sync, nc.scalar, nc.sync, nc.scalar)[
                        sub if spread else 0
                    ]
                    q_eng.dma_start(
                        out=y[ts(4 * qc + sub, 128), :], in_=o_sb[:, sub, :]
                    )

                prev = None  # (qc, zp) awaiting normalize
                for qc in range(QC):
                    zp = psZ.tile(
                        [E + 1, 512], F32, tag="zacc", name=f"zacc{qc}"
                    )
                    pending = []  # (p_tile, ka, kb) AVs deferred two iters
                    for kp in range(KP):
                        if prev is not None:
                            if kp == 0:
                                norm_start(*prev)
                            elif kp in (3, 6, 9, 12):
                                norm_step(prev[0], kp // 3 - 1, psN)
                        ka, kb = 2 * kp, 2 * kp + 1
                        sp = psB.tile(
                            [128, 2, 512], F32, tag="spair", name=f"sp{qc}_{kp}"
                        )
                        nc.tensor.matmul(
                            sp[:, 0, :],
                            krep[:, ts(ka, 128)],
                            qtpair[0:64, ts(qc, 512)],
                            start=True,
                            stop=True,
                        )
                        nc.tensor.matmul(
                            sp[:, 1, :],
                            kv_sb[64:128, ts(kb, 128)],
                            qtpair[64:128, ts(qc, 512)],
                            start=True,
                            stop=True,
                        )
                        p_sb = ppool.tile(
                            [128, 2, 512], FP8, tag="p", name=f"p{qc}_{kp}"
                        )
                        # full-pair exp alternates engines: ScalarE native
                        # Exp on even kp, VectorE Schraudolph on odd kp.
                        # kps 0-2 all go to ScalarE: it is idle at the qc
                        # boundary (DVE drains the Z-evacuation copy), and
                        # an early exp(kp0) completion unblocks the psB
                        # recycle that otherwise stalls the kp3 scores.
                        if kp < 3 or kp % 2 == 0:
                            nc.scalar.activation(
                                p_sb[:, :, :], sp[:, :, :],
                                mybir.ActivationFunctionType.Exp, scale=SCALE,
                            )
                        else:
                            nc.vector.tensor_scalar(
                                p_sb[:, :, :].bitcast(INT8), sp[:, :, :],
                                SCH_A, SCH_B,
                                op0=mybir.AluOpType.mult,
                                op1=mybir.AluOpType.add,
                            )
                        if len(pending) == 3:
                            pp_, pka, pkb = pending.pop(0)
                            nc.tensor.matmul(
                                zp, vp_sb[:, pka:pka + 2, 0:E + 1],
                                pp_[:, :, :],
                                start=(pka == 0), stop=False,
                                perf_mode=mybir.MatmulPerfMode.DoubleRow,
                            )
                        pending.append((p_sb, ka, kb))
                    for pp_, pka, pkb in pending:
                        nc.tensor.matmul(
                            zp, vp_sb[:, pka:pka + 2, 0:E + 1], pp_[:, :, :],
                            start=False, stop=(pkb == KT - 1),
                            perf_mode=mybir.MatmulPerfMode.DoubleRow,
                        )
                    prev = (qc, zp)
                # final chunk's normalize in the tail
                norm_start(*prev)
            with tc.tile_pool(name="psT", bufs=4, space="PSUM") as psT:
                for sub in range(4):
                    norm_step(prev[0], sub, psT, spread=True)
    nc.compile()
    return nc


def get_nc():
    if "nc" not in _NC_CACHE:
        _NC_CACHE["nc"] = build_nc()
    return _NC_CACHE["nc"]


def make_in_maps(x, Wq, bq, Wk, bk, Wv, bv):
    x = np.asarray(x, dtype=np.float32)
    Wq = np.asarray(Wq, dtype=np.float32)
    Wk = np.asarray(Wk, dtype=np.float32)
    Wv = np.asarray(Wv, dtype=np.float32)
    bq = np.asarray(bq, dtype=np.float32)
    bk = np.asarray(bk, dtype=np.float32)
    bv = np.asarray(bv, dtype=np.float32)

    wkv_host = np.ascontiguousarray(
        np.concatenate([Wv.T, Wk.T], axis=1)
    ).astype(NP_BF16)
    wqq_host = np.ascontiguousarray(
        np.concatenate([Wq.T, Wq.T], axis=1)
    ).astype(NP_BF16)
    bkv_host = np.ascontiguousarray(
        np.concatenate([bv, bk]).reshape(128, 1)
    ).astype(np.float32)
    bqq_host = np.ascontiguousarray(
        np.concatenate([bq, bq]).reshape(128, 1)
    ).astype(np.float32)

    in_maps = []
    for c in range(8):
        b, h = c // 2, c % 2
        xT_b = np.asarray(x[b].T, dtype=NP_BF16)
        if h == 1:  # put this core's query half into columns [0, 2048)
            xT_b = np.concatenate([xT_b[:, TQ:], xT_b[:, :TQ]], axis=1)
        in_maps.append(
            {
                "xT": np.ascontiguousarray(xT_b),
                "wqq": wqq_host,
                "wkv": wkv_host,
                "bqq": bqq_host,
                "bkv": bkv_host,
            }
        )
    return in_maps


def assemble(results):
    out = np.empty((B, S_FULL, E), dtype=np.float32)
    for c in range(8):
        b, h = c // 2, c % 2
        out[b, h * TQ:(h + 1) * TQ, :] = results[c]["y"]
    return out


def kernel(x, Wq, bq, Wk, bk, Wv, bv, **_unused):
    in_maps = make_in_maps(x, Wq, bq, Wk, bk, Wv, bv)
    nc = get_nc()
    res = run_bass_kernel_spmd(nc, in_maps, core_ids=list(range(8)))
    return assemble(res.results)


# revision 39
# speedup vs baseline: 1.0351x; 1.0248x over previous
"""Attention (B=4, S=4096, W=512, E=64) on 8 TRN2 NeuronCores.

Sharding: core c handles batch b = c//2, query half h = c%2 (2048 queries).
Each core receives x[b]^T as bf16 with the key/value columns ordered so that
this core's query half occupies columns [0, 2048) (softmax over keys is
permutation invariant as long as K and V share the order, so odd cores get
the two halves swapped). K/V are computed for the full sequence locally; a
flash-style attention runs over the core's query half. No collectives
(pair-wise AllGather was measured at ~17us per op in this stack - slower
than just duplicating the K/V projection on both cores of a pair).

Per-core dataflow (bf16 scores, fp8e4 AV with DoubleRow):
  x^T [512,4096] --[Wv|Wk] pass--> kv = V^T (p0:64) / K^T (p64:128)
  x^T[:, :2048] --[Wq|Wq] pass--> Q^T duplicated on both partition halves
  K^T replicated to partitions 0:64 via SBUF->SBUF DMA on the sync
  queue (the SP engine is idle after the x loads; a gpsimd-queue DMA
  would cost a ~2us dge drain at teardown, and an Act-queue trigger
  would stall ScalarE on the bias-add semaphores)
  V' = [V | 1 | pad-to-80] tiles in fp8e4 via PE transpose, interleaved
  with the projection chunks to ride the x-DMA gaps
  scores: S^T[k,q] = K^T.T @ Q^T, two k-tiles run CONCURRENTLY in the two
  64-row PE row groups (~386ns per pair)
  P = exp(S^T/8) -> fp8e4, one full [128,2x512] instruction per k-pair,
  alternating engines: even kp native Exp on ScalarE, odd kp on VectorE
  via Schraudolph (bits = s*SCH_A + SCH_B stored int8; that bit pattern
  IS the e4m3 encoding of exp(s/8), since e4m3 has 8 codes per octave)
  Z'^T[e+1,q] += V'.T @ P as ONE fp8 DoubleRow matmul per k-pair (~405ns
  for 256 contraction rows; V' ones column accumulates the denominator)
  normalize per query chunk: PE-transpose Z'^T, reciprocal on VectorE,
  scale on ScalarE, per-sub DMA out - overlapped with the next sweep.

PSUM: projection pool 6 banks (closes before the loop; fewer buffers
starve the PE behind the bias-add/V'-copy round trips), then Z
accumulator 1 bank + norm scratch 1 bank + triple-buffered score pairs
6 banks.  AV matmuls are emitted three iterations late so the PE's
in-order stream never waits on exp; exp of pair k frees its PSUM banks
before the scores of pair k+3 need them.
"""

import numpy as np
import ml_dtypes

import concourse.bass as bass
import concourse.mybir as mybir
import concourse.tile as tile
from concourse import bacc
from concourse.bass import ts
from concourse.masks import make_identity
from concourse.bass_utils import run_bass_kernel_spmd

BF16 = mybir.dt.bfloat16
F32 = mybir.dt.float32
FP8 = mybir.dt.float8e4
INT8 = mybir.dt.int8
NP_BF16 = ml_dtypes.bfloat16

# Schraudolph exp on DVE: bits = s * SCH_A + SCH_B, stored int8, bitcast
# fp8e4.  SCH_A folds the 1/sqrt(E) softmax scale and log2(e) into the
# e4m3 exponent step (8 codes per octave); SCH_B centers on the exponent
# bias (7*8) minus the mean log error of the mantissa interpolation.
# Verified on hardware: the DVE float->int8 store rounds to nearest.
SCH_A = 0.125 * 8 * 1.4426950408889634
SCH_B = 56.0 - 0.458

B = 4
S_FULL = 4096
W = 512
E = 64
TQ = 2048  # queries per core
WT = W // 128  # 4 contraction tiles
KT = S_FULL // 128  # 32 key tiles
KP = KT // 2  # 16 key-tile pairs
QC = TQ // 512  # 4 query chunks of 512
NCH = S_FULL // 512  # 8 projection chunks
SCALE = 0.125  # 1/sqrt(E)

_NC_CACHE = {}


def build_nc():
    nc = bacc.Bacc("TRN2", target_bir_lowering=False)
    xT = nc.dram_tensor("xT", [W, S_FULL], BF16, kind="ExternalInput")
    wqq = nc.dram_tensor("wqq", [W, 128], BF16, kind="ExternalInput")
    wkv = nc.dram_tensor("wkv", [W, 128], BF16, kind="ExternalInput")
    bqq = nc.dram_tensor("bqq", [128, 1], F32, kind="ExternalInput")
    bkv = nc.dram_tensor("bkv", [128, 1], F32, kind="ExternalInput")
    y = nc.dram_tensor("y", [TQ, E], F32, kind="ExternalOutput")

    with tile.TileContext(nc) as tc:
        with (
            tc.tile_pool(name="const", bufs=1) as const,
            tc.tile_pool(name="psZ", bufs=1, space="PSUM") as psZ,
            tc.tile_pool(name="pp", bufs=8) as ppool,
            tc.tile_pool(name="zsb", bufs=2) as zsbp,
            tc.tile_pool(name="small", bufs=2) as small,
            tc.tile_pool(name="outp", bufs=2) as outp,
        ):
            # weights/biases as single HWDGE DMAs ahead of the x^T stream
            wqq_sb = const.tile([128, WT, 128], BF16)
            wkv_sb = const.tile([128, WT, 128], BF16)
            nc.scalar.dma_start(
                out=wkv_sb, in_=wkv[:, :].rearrange("(t p) m -> p t m", t=WT)
            )
            nc.scalar.dma_start(
                out=wqq_sb, in_=wqq[:, :].rearrange("(t p) m -> p t m", t=WT)
            )
            bqq_sb = const.tile([128, 1], F32)
            bkv_sb = const.tile([128, 1], F32)
            nc.scalar.dma_start(out=bkv_sb, in_=bkv[:, :])
            nc.scalar.dma_start(out=bqq_sb, in_=bqq[:, :])

            # x^T streamed in per 1024-column block (HWDGE)
            xt_sb = const.tile([128, WT, S_FULL], BF16)
            for ch2 in range(NCH // 2):
                for t in range(WT):
                    nc.sync.dma_start(
                        out=xt_sb[:, t, ts(ch2, 1024)],
                        in_=xT[t * 128:(t + 1) * 128, ts(ch2, 1024)],
                    )

            ident_bf = const.tile([64, 64], BF16)
            make_identity(nc, ident_bf)
            ident_f32 = const.tile([E + 1, E + 1], F32)
            make_identity(nc, ident_f32)

            kv_sb = const.tile([128, S_FULL], BF16)  # V^T (p0:64) / K^T (p64:)
            krep = const.tile([64, S_FULL], BF16)  # K^T replica on p0:64
            qtpair = const.tile([128, TQ], BF16)  # Q^T on both halves
            # inner dim padded 65->80 so the DoubleRow pair stride is
            # 16B-aligned
            vp_sb = const.tile([128, KT, 80], FP8)  # V' = [V | 1 | pad]
            nc.vector.memset(vp_sb, 1.0)

            with tc.tile_pool(name="psA", bufs=6, space="PSUM") as psA:
                def emit_kv_proj(ch):
                    # K/V projection chunk; one fused bias add on VectorE
                    ps = psA.tile([128, 512], F32, tag="mm", name=f"pskv{ch}")
                    for t in range(WT):
                        nc.tensor.matmul(
                            ps,
                            wkv_sb[:, t, :],
                            xt_sb[:, t, ts(ch, 512)],
                            start=(t == 0),
                            stop=(t == WT - 1),
                        )
                    nc.vector.tensor_scalar_add(
                        kv_sb[:, ts(ch, 512)], ps, bkv_sb
                    )
                    nc.sync.dma_start(
                        out=krep[:, ts(ch, 512)], in_=kv_sb[64:128, ts(ch, 512)]
                    )

                def emit_vtrans1(kt_i):
                    vt_ps = psA.tile(
                        [128, E], BF16, tag="mm", name=f"vtps{kt_i}"
                    )
                    nc.tensor.transpose(
                        vt_ps, kv_sb[0:64, ts(kt_i, 128)], ident_bf
                    )
                    # PSUM bf16 -> SBUF fp8; alternate engines to balance
                    if kt_i % 2 == 0:
                        nc.vector.tensor_copy(vp_sb[:, kt_i, 0:E], vt_ps)
                    else:
                        nc.scalar.copy(vp_sb[:, kt_i, 0:E], vt_ps)

                def emit_q_chunk(ch):
                    psq = psA.tile([128, 512], F32, tag="mm", name=f"psq{ch}")
                    for t in range(WT):
                        nc.tensor.matmul(
                            psq,
                            wqq_sb[:, t, :],
                            xt_sb[:, t, ts(ch, 512)],
                            start=(t == 0),
                            stop=(t == WT - 1),
                        )
                    # Q bias on ScalarE: DVE carries the K/V biases and
                    # fp8 copies, so its projection tail otherwise delays
                    # qc0's first Schraudolph exps
                    nc.scalar.activation(
                        qtpair[:, ts(ch, 512)], psq,
                        mybir.ActivationFunctionType.Identity, bias=bqq_sb,
                    )

                emit_kv_proj(0)
                emit_q_chunk(0)
                emit_kv_proj(1)
                for kt_i in range(0, 4):
                    emit_vtrans1(kt_i)
                emit_q_chunk(1)
                emit_kv_proj(2)
                for kt_i in range(4, 8):
                    emit_vtrans1(kt_i)
                emit_q_chunk(2)
                emit_kv_proj(3)
                for kt_i in range(8, 12):
                    emit_vtrans1(kt_i)
                emit_q_chunk(3)
                for ch in range(4, NCH):
                    emit_kv_proj(ch)
                    for kt_i in range(4 * ch - 4, 4 * ch):
                        emit_vtrans1(kt_i)
                for kt_i in range(28, 32):
                    emit_vtrans1(kt_i)

            with (
                tc.tile_pool(name="psB", bufs=3, space="PSUM") as psB,
                tc.tile_pool(name="psN", bufs=1, space="PSUM") as psN,
            ):
                norm_state = {}

                def norm_start(qc, zp, split=False):
                    zsb = zsbp.tile(
                        [E + 1, 512], F32, tag="zsb", name=f"zsb{qc}"
                    )
                    if split:  # tail: both engines idle, halve the latency
                        nc.vector.tensor_copy(zsb[:, 0:256], zp[:, 0:256])
                        nc.scalar.copy(zsb[:, 256:512], zp[:, 256:512])
                    else:
                        nc.vector.tensor_copy(zsb, zp)
                    o_sb = outp.tile([128, 4, E], F32, tag="o", name=f"osb{qc}")
                    norm_state[qc] = (zsb, o_sb)

                def norm_step(qc, sub, pool, spread=False):
                    zsb, o_sb = norm_state[qc]
                    zt = pool.tile(
                        [128, E + 1], F32, tag="zt", name=f"zt{qc}_{sub}"
                    )
                    nc.tensor.transpose(zt, zsb[:, ts(sub, 128)], ident_f32)
                    r = small.tile([128, 1], F32, tag="r", name=f"r{qc}_{sub}")
                    nc.vector.reciprocal(r, zt[:, E:E + 1])
                    if spread and sub % 2 == 1:
                        nc.vector.tensor_scalar_mul(
                            o_sb[:, sub, :], zt[:, 0:E], r
                        )
                    else:
                        nc.scalar.mul(o_sb[:, sub, :], zt[:, 0:E], r)
                    q_eng = (nc.sync, nc.scalar, nc.sync, nc.scalar)[
                        sub if spread else 0
                    ]
                    q_eng.dma_start(
                        out=y[ts(4 * qc + sub, 128), :], in_=o_sb[:, sub, :]
                    )

                prev = None  # (qc, zp) awaiting normalize
                for qc in range(QC):
                    zp = psZ.tile(
                        [E + 1, 512], F32, tag="zacc", name=f"zacc{qc}"
                    )
                    pending = []  # (p_tile, ka, kb) AVs deferred two iters
                    for kp in range(KP):
                        if prev is not None:
                            if kp == 0:
                                norm_start(*prev)
                            elif kp in (3, 7, 9, 13):
                                norm_step(
                                    prev[0], (3, 7, 9, 13).index(kp), psN
                                )
                        ka, kb = 2 * kp, 2 * kp + 1
                        sp = psB.tile(
                            [128, 2, 512], F32, tag="spair", name=f"sp{qc}_{kp}"
                        )
                        nc.tensor.matmul(
                            sp[:, 0, :],
                            krep[:, ts(ka, 128)],
                            qtpair[0:64, ts(qc, 512)],
                            start=True,
                            stop=True,
                        )
                        nc.tensor.matmul(
                            sp[:, 1, :],
                            kv_sb[64:128, ts(kb, 128)],
                            qtpair[64:128, ts(qc, 512)],
                            start=True,
                            stop=True,
                        )
                        p_sb = ppool.tile(
                            [128, 2, 512], FP8, tag="p", name=f"p{qc}_{kp}"
                        )
                        # full-pair exp alternates engines: ScalarE native
                        # Exp on even kp, VectorE Schraudolph on odd kp.
                        # kps 0-2 all go to ScalarE: it is idle at the qc
                        # boundary (DVE drains the Z-evacuation copy), and
                        # an early exp(kp0) completion unblocks the psB
                        # recycle that otherwise stalls the kp3 scores.
                        if kp < 3 or kp % 2 == 0:
                            nc.scalar.activation(
                                p_sb[:, :, :], sp[:, :, :],
                                mybir.ActivationFunctionType.Exp, scale=SCALE,
                            )
                        else:
                            nc.vector.tensor_scalar(
                                p_sb[:, :, :].bitcast(INT8), sp[:, :, :],
                                SCH_A, SCH_B,
                                op0=mybir.AluOpType.mult,
                                op1=mybir.AluOpType.add,
                            )
                        if len(pending) == 3:
                            pp_, pka, pkb = pending.pop(0)
                            nc.tensor.matmul(
                                zp, vp_sb[:, pka:pka + 2, 0:E + 1],
                                pp_[:, :, :],
                                start=(pka == 0), stop=False,
                                perf_mode=mybir.MatmulPerfMode.DoubleRow,
                            )
                        pending.append((p_sb, ka, kb))
                    for pp_, pka, pkb in pending:
                        nc.tensor.matmul(
                            zp, vp_sb[:, pka:pka + 2, 0:E + 1], pp_[:, :, :],
                            start=False, stop=(pkb == KT - 1),
                            perf_mode=mybir.MatmulPerfMode.DoubleRow,
                        )
                    prev = (qc, zp)
                # final chunk's normalize in the tail
                norm_start(*prev, split=True)
            with tc.tile_pool(name="psT", bufs=4, space="PSUM") as psT:
                for sub in range(4):
                    norm_step(prev[0], sub, psT, spread=True)
    nc.compile()
    return nc


def get_nc():
    if "nc" not in _NC_CACHE:
        _NC_CACHE["nc"] = build_nc()
    return _NC_CACHE["nc"]


def make_in_maps(x, Wq, bq, Wk, bk, Wv, bv):
    x = np.asarray(x, dtype=np.float32)
    Wq = np.asarray(Wq, dtype=np.float32)
    Wk = np.asarray(Wk, dtype=np.float32)
    Wv = np.asarray(Wv, dtype=np.float32)
    bq = np.asarray(bq, dtype=np.float32)
    bk = np.asarray(bk, dtype=np.float32)
    bv = np.asarray(bv, dtype=np.float32)

    wkv_host = np.ascontiguousarray(
        np.concatenate([Wv.T, Wk.T], axis=1)
    ).astype(NP_BF16)
    wqq_host = np.ascontiguousarray(
        np.concatenate([Wq.T, Wq.T], axis=1)
    ).astype(NP_BF16)
    bkv_host = np.ascontiguousarray(
        np.concatenate([bv, bk]).reshape(128, 1)
    ).astype(np.float32)
    bqq_host = np.ascontiguousarray(
        np.concatenate([bq, bq]).reshape(128, 1)
    ).astype(np.float32)

    in_maps = []
    for c in range(8):
        b, h = c // 2, c % 2
        xT_b = np.asarray(x[b].T, dtype=NP_BF16)
        if h == 1:  # put this core's query half into columns [0, 2048)
            xT_b = np.concatenate([xT_b[:, TQ:], xT_b[:, :TQ]], axis=1)
        in_maps.append(
            {
                "xT": np.ascontiguousarray(xT_b),
                "wqq": wqq_host,
                "wkv": wkv_host,
                "bqq": bqq_host,
                "bkv": bkv_host,
            }
        )
    return in_maps


def assemble(results):
    out = np.empty((B, S_FULL, E), dtype=np.float32)
    for c in range(8):
        b, h = c // 2, c % 2
        out[b, h * TQ:(h + 1) * TQ, :] = results[c]["y"]
    return out


def kernel(x, Wq, bq, Wk, bk, Wv, bv, **_unused):
    in_maps = make_in_maps(x, Wq, bq, Wk, bk, Wv, bv)
    nc = get_nc()
    res = run_bass_kernel_spmd(nc, in_maps, core_ids=list(range(8)))
    return assemble(res.results)
